# revision 167
# baseline (speedup 1.0000x reference)
"""DaGCN on 8 Trainium2 NeuronCores (Bass SPMD).

Strategy (graph/data parallel, nodes sharded 8 ways):
  * Each core owns a 6250-node shard (padded to 6272 = 49*128).
  * Feature transforms s = x @ W run as bf16 PE matmuls on host-transposed
    x shards; the resulting per-shard tables are AllGather'ed so every core
    holds the full [50176, 128] bf16 node-feature tables in its DRAM.
  * Edges are assigned to the core owning dst. Per (adjacency, src-half)
    they are sorted by dst block (128 nodes), each block's run padded to
    whole 128-edge chunks. dma_gather (1024 idxs/call) fetches s[src] as
    bf16 messages, edge-on-partition.
  * segment_sum runs on the TensorEngine: per 128-edge chunk a one-hot
    lhsT [128 edges x 128 dst-cols] holding ew (built on DVE from an iota
    compare) is matmul'ed with the message chunk, accumulating each dst
    block in PSUM. No scatter-add (HW races on duplicate indices).
  * Gating/normalization math runs on DVE/ACT over [128, 49, F] shard
    layouts entirely in SBUF.
"""

import math
from contextlib import ExitStack

import ml_dtypes
import numpy as np

import concourse.bacc as bacc
import concourse.bass as bass
import concourse.mybir as mybir
from concourse.bass_utils import run_bass_kernel_spmd

F32 = mybir.dt.float32
BF16 = mybir.dt.bfloat16
I16 = mybir.dt.int16
AOP = mybir.AluOpType
ACT = mybir.ActivationFunctionType

NCORES = 8
N = 50000
NFEAT, NHID, NCLASS = 256, 64, 32
S_CALL = 1024          # idxs per dma_gather call (HW-validated; 2048 hangs)
CALL_CHUNKS = S_CALL // 128
RING = 8               # gather/onehot ring depth (in calls)
NPSUM = 6              # psum block-accumulator ring


def _wrap16(a):
    """[n] int16 -> [128, n//16]: idx i at [i%16, i//16], replicated x8."""
    n = a.shape[0]
    w = a.reshape(n // 16, 16).T.astype(np.int16)
    return np.tile(w, (8, 1)).copy()


def _chunkwrap(a, dtype):
    """[n] -> [128, n//128]: edge i at [i%128, i//128]."""
    n = a.shape[0]
    return np.ascontiguousarray(a.reshape(n // 128, 128).T.astype(dtype))


def _balance(inputs, SP, NB):
    """Assign nodes to (core, block, col) slots so per-block in-degrees are
    balanced for both adjacencies: deal nodes round-robin (snaking) in
    descending total-degree order, sorting each round by adjacency-1 degree."""
    d1 = np.bincount(np.asarray(inputs["dst1"]).astype(np.int64), minlength=N)
    d2 = np.bincount(np.asarray(inputs["dst2"]).astype(np.int64), minlength=N)
    NBLK = NCORES * NB
    order = np.argsort(-(d1 + d2), kind="stable")
    node2slot = np.full(N, -1, np.int64)
    slot_fill = np.zeros(NBLK, np.int64)
    blocks = np.arange(NBLK)
    pos, r = 0, 0
    while pos < N:
        take = min(NBLK, N - pos)
        grp = order[pos:pos + take]
        grp = grp[np.argsort(-d1[grp], kind="stable")]
        tgt = (blocks if r % 2 == 0 else blocks[::-1])[:take]
        node2slot[grp] = tgt * 128 + slot_fill[tgt]
        slot_fill[tgt] += 1
        pos += take
        r += 1
    slot2node = np.full(NCORES * SP, -1, np.int64)
    slot2node[node2slot] = np.arange(N)
    return node2slot, slot2node


def _prep_adjacency(src, dst, ew, node2slot, S, SP, NB, HSPLIT, NROWS):
    """Bucket edges by dst core/block/src-half; returns per-core arrays + CPBs."""
    src = np.asarray(src).astype(np.int64)
    dst = np.asarray(dst).astype(np.int64)
    ew = np.asarray(ew).astype(np.float32)
    dsl = node2slot[dst]
    core = dsl // SP
    row = node2slot[src]                    # padded table row
    half = (row >= HSPLIT).astype(np.int64)
    dstrel = dsl - core * SP
    blk = dstrel // 128
    col = dstrel % 128

    percore = []
    counts = np.zeros((NCORES, 2, NB), np.int64)
    for k in range(NCORES):
        m = core == k
        e = np.lexsort((blk[m], half[m]))   # sort by (half, block)
        r, h, b, c, w = row[m][e], half[m][e], blk[m][e], col[m][e], ew[m][e]
        percore.append((r, h, b, c, w))
        for hh in range(2):
            mm = h == hh
            counts[k, hh] = np.bincount(b[mm], minlength=NB)

    cpb_lo = int(np.ceil(counts[:, 0].max() / 128))
    cpb_hi = int(np.ceil(counts[:, 1].max() / 128))
    cpb_lo = max(cpb_lo, 1)
    cpb_hi = max(cpb_hi, 1)
    ch_lo = -(-NB * cpb_lo // CALL_CHUNKS) * CALL_CHUNKS
    ch_hi = -(-NB * cpb_hi // CALL_CHUNKS) * CALL_CHUNKS
    nslot = (ch_lo + ch_hi) * 128

    out = []
    for k in range(NCORES):
        r, h, b, c, w = percore[k]
        gidx = np.zeros(nslot, np.int64)
        dcol = np.zeros(nslot, np.int64)
        eww = np.zeros(nslot, np.float32)
        for hh, cpb, base_ch, rowbase in ((0, cpb_lo, 0, 0), (1, cpb_hi, ch_lo, HSPLIT)):
            mm = h == hh
            rr, bb, cc, ww = r[mm], b[mm], c[mm], w[mm]
            # position within block run (edges already sorted by block)
            cnt = counts[k, hh]
            offs = np.concatenate(([0], np.cumsum(cnt)))[:-1]
            pos = np.arange(rr.shape[0]) - offs[bb]
            slot = (base_ch + bb * cpb) * 128 + pos
            gidx[slot] = rr - rowbase
            dcol[slot] = cc
            eww[slot] = ww
        out.append((
            _wrap16(gidx),
            _chunkwrap(dcol, np.float32),
            _chunkwrap(eww, np.float32),
        ))
    return out, cpb_lo, cpb_hi, ch_lo, ch_hi, nslot


class Ctr:
    def __init__(self, sem, step=1):
        self.sem, self.n, self.step = sem, 0, step

    def inc(self, inst):
        inst.then_inc(self.sem, self.step)
        self.n += self.step
        return self.n


def _build(S, SP, NB, NROWS, HSPLIT, adjmeta, scalars):
    """adjmeta: {a: (ch_lo, ch_hi, nslot)}; scalars: g1b,g2b,h1b,h2b floats."""
    nc = bacc.Bacc("TRN2", num_devices=NCORES, num_swdge_queues=2)
    g1b, g2b, h1b, h2b = scalars
    ncal_max = max((m[0] + m[1]) // CALL_CHUNKS for m in adjmeta.values())
    nslot_max = max(m[2] for m in adjmeta.values())
    nch_max = nslot_max // 128

    # ---------------- I/O ----------------
    din = {}
    for v in ("xt1a", "xt1b", "xt2a", "xt2b"):
        din[v] = nc.dram_tensor(v, [128, 2, SP], BF16, kind="ExternalInput")
    din["w1a"] = nc.dram_tensor("w1a", [128, 2, NHID], BF16, kind="ExternalInput")
    din["w1b"] = nc.dram_tensor("w1b", [128, 2, NHID], BF16, kind="ExternalInput")
    din["w2"] = nc.dram_tensor("w2", [128, 128], BF16, kind="ExternalInput")
    din["iota"] = nc.dram_tensor("iota", [128, 128], BF16, kind="ExternalInput")
    din["idf"] = nc.dram_tensor("idf", [128, 128], F32, kind="ExternalInput")
    din["idb"] = nc.dram_tensor("idb", [128, 128], BF16, kind="ExternalInput")
    din["g1w"] = nc.dram_tensor("g1w", [128, 128], BF16, kind="ExternalInput")
    din["g2w"] = nc.dram_tensor("g2w", [128, 128], BF16, kind="ExternalInput")
    din["h1w"] = nc.dram_tensor("h1w", [128, 64], BF16, kind="ExternalInput")
    din["h2w"] = nc.dram_tensor("h2w", [128, 64], BF16, kind="ExternalInput")
    din["b1r"] = nc.dram_tensor("b1r", [128, 128], BF16, kind="ExternalInput")
    din["b2r"] = nc.dram_tensor("b2r", [128, 64], BF16, kind="ExternalInput")
    for a in (1, 2):
        ns = adjmeta[a][2]
        din[f"gidx{a}"] = nc.dram_tensor(f"gidx{a}", [128, ns // 16], I16, kind="ExternalInput")
        din[f"dst{a}"] = nc.dram_tensor(f"dst{a}", [128, ns // 128], F32, kind="ExternalInput")
        din[f"eww{a}"] = nc.dram_tensor(f"eww{a}", [128, ns // 128], F32, kind="ExternalInput")
    out_o = nc.dram_tensor("out_o", [SP, NCLASS], F32, kind="ExternalOutput")
    p1_o = nc.dram_tensor("p1_o", [SP, NCLASS], F32, kind="ExternalOutput")
    p2_o = nc.dram_tensor("p2_o", [SP, NCLASS], F32, kind="ExternalOutput")

    t_in = {t: nc.dram_tensor(f"t{t}in", [SP, 128], BF16) for t in (1, 2, 3)}
    t_full = {t: nc.dram_tensor(f"t{t}full", [NROWS, 128], BF16,
                                addr_space="Shared")
              for t in (1, 2, 3)}

    ctx = ExitStack()
    sb = lambda name, shape, dt: ctx.enter_context(nc.sbuf_tensor(name, shape, dt))
    ps = lambda name, shape: ctx.enter_context(nc.psum_tensor(name, shape, F32))
    sem = lambda name: ctx.enter_context(nc.semaphore(name))

    # ---------------- SBUF ----------------
    c_w1a = sb("c_w1a", [128, 2, NHID], BF16)
    c_w1b = sb("c_w1b", [128, 2, NHID], BF16)
    c_w2 = sb("c_w2", [128, 128], BF16)
    c_iota = sb("c_iota", [128, 128], BF16)
    c_idf = sb("c_idf", [128, 128], F32)
    c_idb = sb("c_idb", [128, 128], BF16)
    c_g1w = sb("c_g1w", [128, 128], BF16)
    c_g2w = sb("c_g2w", [128, 128], BF16)
    c_h1w = sb("c_h1w", [128, 64], BF16)
    c_h2w = sb("c_h2w", [128, 64], BF16)
    c_b1r = sb("c_b1r", [128, 128], BF16)
    c_b2r = sb("c_b2r", [128, 64], BF16)

    stg = sb("stg", [128, 16, 128], BF16)  # table staging: 2 groups of 8 blocks
    agg1 = sb("agg1", [128, NB, 128], BF16)
    agg2 = sb("agg2", [128, NB, 128], BF16)
    tmp = sb("tmp", [128, NB, 128], BF16)
    tmp2 = sb("tmp2", [128, NB - 27, 128], BF16)  # Pool-branch gating scratch
    xtt = sb("xtt", [128, 2, 128], BF16)
    lamv = {nm: sb(nm, [128, NB], F32)
            for nm in ("l1", "l2", "lsum", "w0", "w1")}
    cbias = sb("cbias", [128, 4], F32)
    # per-adjacency edge metadata, loaded once and reused by both layers
    gidx_sb = {a: sb(f"gidx{a}_sb", [128, adjmeta[a][2] // 16], I16)
               for a in (1, 2)}
    dst_sb = {a: sb(f"dst{a}_sb", [128, adjmeta[a][2] // 128], F32)
              for a in (1, 2)}
    ew_sb = {a: sb(f"ew{a}_sb", [128, adjmeta[a][2] // 128], F32)
             for a in (1, 2)}
    sbA = ExitStack()
    xta = sbA.enter_context(nc.sbuf_tensor("xta", [128, 2, SP], BF16))
    xtb = sbA.enter_context(nc.sbuf_tensor("xtb", [128, 2, SP], BF16))
    sT = sbA.enter_context(nc.sbuf_tensor("sT", [128, SP], BF16))

    psA = ExitStack()
    mm_ps = [psA.enter_context(nc.psum_tensor(f"mm_ps{i}", [128, 512], F32))
             for i in range(2)]
    trb_ps = [psA.enter_context(nc.psum_tensor(f"trb_ps{i}", [128, 128], BF16))
              for i in range(2)]

    io = Ctr(sem("io"), 16)        # sync-engine DMAs
    pio = Ctr(sem("pio"), 16)      # Pool software-DGE DMAs
    gpv = Ctr(sem("gpv"), 1)       # Pool compute milestones
    gsems = [Ctr(sem(f"g{i}"), 16) for i in range(RING)]  # per-ring-slot gathers
    ccs = [Ctr(sem(f"cc{i}"), 1) for i in range(3)]   # one sem per collective
    pe = Ctr(sem("pe"), 1)         # PE milestones
    dv = Ctr(sem("dv"), 1)         # DVE milestones
    ac = Ctr(sem("ac"), 1)         # ACT milestones

    SY, PE, DV, AC, GP = nc.sync, nc.tensor, nc.vector, nc.scalar, nc.gpsimd

    def fence():
        # sync engine waits for all its issued DMAs: later cross-engine
        # io-threshold waits become unambiguous (no completion reordering).
        SY.wait_ge(io.sem, io.n)

    # =========== Phase A: constants + s tables ===========
    for bi, bval in enumerate((g1b, g2b, h1b, h2b)):
        nc.vector.memset(cbias[:, bi:bi + 1], float(bval))
    # only the matmul/transpose constants block the critical path; the rest
    # load on the ACT ring after the x pumps (see below)
    for name, t in (("w1a", c_w1a), ("w1b", c_w1b), ("idb", c_idb)):
        io.inc(SY.dma_start(t[:], din[name][:]))
    consts_io = io.n

    nsl = [(j * 512, min(512, SP - j * 512)) for j in range((SP + 511) // 512)]
    trb_free = {}   # trb bank -> dv val of the copy that freed it
    stg_free = {}   # stg group -> pio val of the batched DMA that freed it
    mmps_free = {}  # mm_ps bank -> dv val of the sT copy that freed it
    aio = Ctr(sem("aio"), 16)   # ACT-ring DMAs (idx loads)
    TBATCH = 8                  # t_in rows per batched DMA (stg is 2 groups)
    # rotating DMA-completion sems: same-sem updates are spaced far apart, so
    # intermediate-threshold waits are deterministic (no overlapping windows)
    NXS = 6
    xsa = [Ctr(sem(f"xsa{i}"), 16) for i in range(NXS)]   # xta slabs (SY)
    xsb = [Ctr(sem(f"xsb{i}"), 16) for i in range(NXS)]   # xtb slabs (ACT)
    sgs = [Ctr(sem(f"sgs{i}"), 16) for i in range(2)]     # t2/t3 flushes (hwdge)
    sgp = [Ctr(sem(f"sgp{i}"), 16) for i in range(2)]     # t1 flushes (Pool swdge)

    def s_table(tbl, va, vb, wa, wb, prev_mm, fe, fs, pre_flush=None):
        """s = [x_va@W1a | x_vb@W1b] -> row-major t_in[tbl], pipelined:
        per-slab x DMAs (xta on SY ring, xtb on ACT ring) -> PE matmul ->
        DVE copy to sT -> per-block PE transpose -> DVE copy into stg ->
        batched row DMA (SY). The x pump runs NXS slabs ahead of PE so each
        slab-class sem is re-incremented only after its waiter passed."""
        xio = [None] * len(nsl)
        mm = []
        copies = []

        def pump(j):
            o, n = nsl[j]
            k = j % NXS
            if j >= NXS:
                SY.wait_ge(pe.sem, mm[j - NXS])
                AC.wait_ge(pe.sem, mm[j - NXS])
            xsa[k].inc(SY.dma_start(xta[:, :, o:o + n], din[va][:, :, o:o + n]))
            xsb[k].inc(AC.dma_start(xtb[:, :, o:o + n], din[vb][:, :, o:o + n]))
            xio[j] = (k, xsa[k].n, xsb[k].n)

        if prev_mm is not None:
            SY.wait_ge(pe.sem, prev_mm[-1])   # xta/xtb + class-sem WAR
            AC.wait_ge(pe.sem, prev_mm[-1])
        for j in range(min(NXS, len(nsl))):
            pump(j)

        def flush_group(t_last):
            """DMA stg group (blocks g*TBATCH..t_last) -> t_in rows."""
            g = t_last // TBATCH
            b0 = g * TBATCH
            nb = t_last - b0 + 1
            s0 = (g % 2) * TBATCH
            if pre_flush is not None and g == 0:
                pre_flush()
            fe.wait_ge(dv.sem, dv.n)
            fs[g % 2].inc(fe.dma_start(
                t_in[tbl][b0 * 128:(b0 + nb) * 128, :]
                .rearrange("(t p) f -> p t f", p=128),
                stg[:, s0:s0 + nb, :]))
            stg_free[g % 2] = (fs[g % 2], fs[g % 2].n)

        def do_blocks(jj):
            for t in range(jj * 4, min((jj + 1) * 4, NB)):
                q = trb_ps[t % 2]
                PE.wait_ge(dv.sem, copies[jj])
                if t % 2 in trb_free:
                    PE.wait_ge(dv.sem, trb_free[t % 2])
                pe.inc(PE.transpose(q[:], sT[:, t * 128:(t + 1) * 128],
                                    c_idb[:]))
                DV.wait_ge(pe.sem, pe.n)
                g = t // TBATCH
                if t % TBATCH == 0 and (g % 2) in stg_free:
                    fctr, fval = stg_free[g % 2]
                    DV.wait_ge(fctr.sem, fval)
                dv.inc(DV.tensor_copy(
                    stg[:, (g % 2) * TBATCH + t % TBATCH, :], q[:]))
                trb_free[t % 2] = dv.n
                if t % TBATCH == TBATCH - 1 or t == NB - 1:
                    flush_group(t)

        for j, (o, n) in enumerate(nsl):
            p = mm_ps[j % 2]
            k, va_, vb_ = xio[j]
            PE.wait_ge(xsa[k].sem, va_)
            PE.wait_ge(xsb[k].sem, vb_)
            if j % 2 in mmps_free:
                PE.wait_ge(dv.sem, mmps_free[j % 2])
            for xt, w, prow in ((xta, wa, 0), (xtb, wb, 64)):
                for cch in range(2):
                    last = PE.matmul(p[prow:prow + 64, 0:n], w[:, cch, :],
                                     xt[:, cch, o:o + n],
                                     start=(cch == 0), stop=(cch == 1))
            pe.inc(last)
            mm.append(pe.n)
            if j + NXS < len(nsl):
                pump(j + NXS)
            DV.wait_ge(pe.sem, pe.n)
            dv.inc(DV.tensor_copy(sT[:, o:o + n], p[:, 0:n]))
            copies.append(dv.n)
            mmps_free[j % 2] = dv.n
            if j >= 1:
                do_blocks(j - 1)
        do_blocks(len(nsl) - 1)
        return ((fs[0], fs[0].n), (fs[1], fs[1].n)), mm

    CCPAGES = 4096  # factor the (contiguous) collective out AP into pages
    cc_done = {}

    def cc_start(ci, t, wait_sgs):
        """AllGather t_in[t] -> t_full[t] on the Pool engine. The out AP is
        re-factored into [pages, page] pairs — the identical contiguous
        region, expressed page-wise."""
        for fctr, fval in wait_sgs:
            GP.wait_ge(fctr.sem, fval)
        cc = GP.collective_compute(
            "AllGather", AOP.bypass,
            replica_groups=[list(range(NCORES))],
            ins=[t_in[t][:]], outs=[t_full[t][:]])
        per = NROWS * 128 // CCPAGES
        cc.ins.outs[0].ap = [[per, CCPAGES], [1, per]]
        ccs[ci].inc(cc)
        cc_done[ci] = ccs[ci].n

    # edge metadata: adjacency 1 on Pool right after cc_t1 (its idle window);
    # adjacency 2 on the SY ring once the x pumps are done
    idx_wait = {}

    def load_idx(a, eng, ctr):
        ctr.inc(eng.dma_start(gidx_sb[a][:], din[f"gidx{a}"][:]))
        ctr.inc(eng.dma_start(dst_sb[a][:], din[f"dst{a}"][:]))
        ctr.inc(eng.dma_start(ew_sb[a][:], din[f"eww{a}"][:]))
        idx_wait[a] = (ctr, ctr.n)

    PE.wait_ge(io.sem, consts_io)   # w1a/w1b/idb loaded
    t1_io, t1_mm = s_table(1, "xt1a", "xt1b", c_w1a, c_w1b, None, GP, sgp)
    cc_start(0, 1, t1_io)
    load_idx(1, GP, pio)
    t2_io, t2_mm = s_table(2, "xt2a", "xt2b", c_w1a, c_w1b, t1_mm, AC, sgs)
    load_idx(2, SY, io)
    # remaining constants on the ACT ring, clear of the pumps
    for name, t in (("iota", c_iota), ("idf", c_idf), ("w2", c_w2),
                    ("g1w", c_g1w), ("g2w", c_g2w), ("h1w", c_h1w),
                    ("h2w", c_h2w), ("b1r", c_b1r), ("b2r", c_b2r)):
        aio.inc(AC.dma_start(t[:], din[name][:]))
    caio = aio.n
    DV.wait_ge(aio.sem, caio)   # c_iota before the first onehot build

    pe_phaseA = pe.n

    # =========== edge pass machinery ===========
    psA.close()  # phase-A PSUM freed; per-engine program order makes reuse safe
    sbA.close()  # xta/xtb freed -> reused by edge buffers (guarded by waits below)
    blk_ps = [ps(f"blk_ps{i}", [128, 128]) for i in range(NPSUM)]
    prop1 = sb("prop1", [128, NB, 64], BF16)
    prop2 = sb("prop2", [128, NB, 64], BF16)
    msg = sb("msg", [128, RING * CALL_CHUNKS, 128], BF16)
    ohr = sb("ohr", [128, RING * CALL_CHUNKS, 128], BF16)
    hstage = sb("hstage", [128, NPSUM, 128], BF16)  # hi-half psum staging
    gcall = [0]      # global gather call counter
    pe_cons_vals = []
    npass = [0]
    psum_last = [None] * NPSUM  # (sem, val) of last copy freeing each psum slot
    BDP = NB - 27    # blocks [0,BDP) combine on Pool; [BDP,NB) on DVE
    hst_free = {}    # hstage slot -> gpv val of the Pool add that freed it

    def psum_wait(E, slot):
        if psum_last[slot] is not None:
            eng, val = psum_last[slot]
            E.wait_ge({"dv": dv, "ac": ac, "gp": gpv}[eng].sem, val)

    def edge_pass(adj, table, F_rhs, dest, cc_need, ch_lo, ch_hi, cpb_lo, cpb_hi,
                  add_mode, inject=None):
        """One (layer, adjacency) pass: lo half then hi half.
        Returns per-block dv marks (hi-half add of block b complete)."""
        gbuf = msg
        gx, dx, ex = gidx_sb[adj], dst_sb[adj], ew_sb[adj]
        local_call = [0]
        deferred = []    # (block, psum slot, ac stage val, call idx)

        def flush_deferred(min_age):
            while deferred and gcall[0] - deferred[0][3] >= min_age:
                b, slot, av, _ = deferred.pop(0)
                GP.wait_ge(ac.sem, av)   # also covers lo_copy_ac[b] (same stream)
                gpv.inc(GP.tensor_tensor(dest[:, b, 0:F_rhs],
                                         dest[:, b, 0:F_rhs],
                                         hstage[:, slot, 0:F_rhs], op=AOP.add))
                hst_free[slot] = ("gp", gpv.n)

        lo_copy_ac = {}
        GP.wait_ge(ccs[cc_need].sem, cc_done[cc_need])
        ictr, ival = idx_wait[adj]
        GP.wait_ge(ictr.sem, ival)
        DV.wait_ge(ictr.sem, ival)
        if npass[0] == 0:
            GP.wait_ge(pe.sem, pe_phaseA)   # msg ring aliases freed xta/xtb
            DV.wait_ge(pe.sem, pe_phaseA)   # ohr ring likewise
        npass[0] += 1
        for half, ch, cpb, base in ((0, ch_lo, cpb_lo, 0), (1, ch_hi, cpb_hi, HSPLIT)):
            ch0 = 0 if half == 0 else ch_lo  # chunk offset in the arrays
            tab = table[base:NROWS] if half == 1 else table[0:HSPLIT]
            blk_of = lambda c: min(c // cpb, NB - 1)
            endc = lambda b: (b + 1) * cpb - 1 if b < NB - 1 else ch - 1
            for j in range(ch // CALL_CHUNKS):
                if inject and local_call[0] in inject:
                    inject[local_call[0]]()
                local_call[0] += 1
                rj = (gcall[0] % RING) * CALL_CHUNKS
                flush_deferred(4)
                if len(pe_cons_vals) >= RING:
                    GP.wait_ge(pe.sem, pe_cons_vals[-RING])
                gslot = gcall[0] % RING
                g = GP.dma_gather(
                    gbuf[:, rj:rj + CALL_CHUNKS, :], tab,
                    gx[:, (ch0 * 8 + j * S_CALL // 16):(ch0 * 8 + (j + 1) * S_CALL // 16)],
                    S_CALL, S_CALL, 128, queue_num=gcall[0] % 2)
                gsems[gslot].inc(g)
                gv = gsems[gslot].n
                # onehot build
                if len(pe_cons_vals) >= RING:
                    DV.wait_ge(pe.sem, pe_cons_vals[-RING])
                cbase = ch0 + j * CALL_CHUNKS
                for c8 in range(CALL_CHUNKS):
                    ts = DV.tensor_scalar(
                        ohr[:, rj + c8, :], c_iota[:],
                        dx[:, cbase + c8:cbase + c8 + 1],
                        ex[:, cbase + c8:cbase + c8 + 1],
                        op0=AOP.is_equal, op1=AOP.mult)
                dv.inc(ts)
                ohv = dv.n
                # matmuls
                PE.wait_ge(gsems[gslot].sem, gv)
                PE.wait_ge(dv.sem, ohv)
                last_was_end = False
                for c8 in range(CALL_CHUNKS):
                    c = j * CALL_CHUNKS + c8
                    b = blk_of(c)
                    slot = b % NPSUM
                    p = blk_ps[slot]
                    st = (c == b * cpb)
                    if st:
                        psum_wait(PE, slot)
                    mmi = PE.matmul(p[:, 0:F_rhs],
                                    ohr[:, rj + c8, :],
                                    gbuf[:, rj + c8, 0:F_rhs],
                                    start=st, stop=(c == endc(b)))
                    last_was_end = (c == endc(b))
                    if last_was_end:
                        pe.inc(mmi)
                        if (add_mode or half == 1) and b < BDP:
                            # ACT stages the psum out; Pool combines (deferred)
                            AC.wait_ge(pe.sem, pe.n)
                            if slot in hst_free:
                                heng, hval = hst_free[slot]
                                AC.wait_ge({"gp": gpv, "dv": dv}[heng].sem,
                                           hval)
                            ac.inc(AC.activation(hstage[:, slot, 0:F_rhs],
                                                 p[:, 0:F_rhs], ACT.Copy))
                            psum_last[slot] = ("ac", ac.n)
                            deferred.append((b, slot, ac.n, gcall[0]))
                        elif add_mode or half == 1:
                            DV.wait_ge(pe.sem, pe.n)
                            DV.wait_ge(ac.sem, lo_copy_ac[b])
                            cpi = DV.tensor_tensor(dest[:, b, 0:F_rhs],
                                                   dest[:, b, 0:F_rhs],
                                                   p[:, 0:F_rhs], op=AOP.add)
                            dv.inc(cpi)
                            psum_last[slot] = ("dv", dv.n)
                        else:
                            AC.wait_ge(pe.sem, pe.n)
                            cpi = AC.activation(dest[:, b, 0:F_rhs],
                                                p[:, 0:F_rhs], ACT.Copy)
                            ac.inc(cpi)
                            psum_last[slot] = ("ac", ac.n)
                            lo_copy_ac[b] = ac.n
                if not last_was_end:
                    pe.inc(mmi)
                pe_cons_vals.append(pe.n)
                gcall[0] += 1
        flush_deferred(0)
        return gpv.n

    m1 = adjmeta[1]
    m2 = adjmeta[2]
    inj1 = {18: lambda: cc_start(1, 2, t2_io)}
    edge_pass(1, t_full[1], 128, agg1, 0, m1[0], m1[1], m1[3], m1[4], False,
              inject=inj1)
    edge_pass(2, t_full[2], 128, agg2, 1, m2[0], m2[1], m2[3], m2[4], False)

    # ====== Phase C: mid gating, block-split Pool [0,BDP) / DVE [BDP,NB) ======
    b1b = c_b1r[:, None, :].broadcast_to([128, NB, 128])
    g1b_b = c_g1w[:, None, :].broadcast_to([128, NB, 128])
    g2b_b = c_g2w[:, None, :].broadcast_to([128, NB, 128])
    X_AX = mybir.AxisListType.X

    GP.wait_ge(aio.sem, caio)
    GP.drain()
    GP.tensor_tensor(agg1[:, 0:BDP], agg1[:, 0:BDP], b1b[:, 0:BDP], op=AOP.add)
    GP.tensor_tensor(agg2[:, 0:BDP], agg2[:, 0:BDP], b1b[:, 0:BDP], op=AOP.add)
    GP.drain()
    GP.tensor_scalar(agg1[:, 0:BDP], agg1[:, 0:BDP], 0.0, None, op0=AOP.max)
    GP.tensor_scalar(agg2[:, 0:BDP], agg2[:, 0:BDP], 0.0, None, op0=AOP.max)
    GP.drain()
    GP.tensor_tensor(tmp[:, 0:BDP], agg1[:, 0:BDP], g1b_b[:, 0:BDP], op=AOP.mult)
    gpv.inc(GP.tensor_tensor(tmp2[:], agg2[:, 0:BDP], g2b_b[:, 0:BDP],
                             op=AOP.mult))
    gp_prod = gpv.n
    DV.drain()
    DV.tensor_tensor(agg1[:, BDP:NB], agg1[:, BDP:NB], b1b[:, BDP:NB], op=AOP.add)
    DV.tensor_tensor(agg2[:, BDP:NB], agg2[:, BDP:NB], b1b[:, BDP:NB], op=AOP.add)
    DV.drain()
    DV.tensor_scalar(agg1[:, BDP:NB], agg1[:, BDP:NB], 0.0, None, op0=AOP.max)
    DV.tensor_scalar(agg2[:, BDP:NB], agg2[:, BDP:NB], 0.0, None, op0=AOP.max)
    DV.drain()
    DV.tensor_tensor(tmp[:, BDP:NB], agg1[:, BDP:NB], g1b_b[:, BDP:NB],
                     op=AOP.mult)
    DV.drain()
    DV.tensor_reduce(lamv["l1"][:, BDP:NB], tmp[:, BDP:NB], axis=X_AX,
                     op=AOP.add)
    DV.drain()
    DV.tensor_tensor(tmp[:, BDP:NB], agg2[:, BDP:NB], g2b_b[:, BDP:NB],
                     op=AOP.mult)
    DV.drain()
    DV.tensor_reduce(lamv["l2"][:, BDP:NB], tmp[:, BDP:NB], axis=X_AX,
                     op=AOP.add)
    DV.wait_ge(gpv.sem, gp_prod)
    DV.tensor_reduce(lamv["l1"][:, 0:BDP], tmp[:, 0:BDP], axis=X_AX, op=AOP.add)
    dv.inc(DV.tensor_reduce(lamv["l2"][:, 0:BDP], tmp2[:], axis=X_AX,
                            op=AOP.add))
    lam_dv = dv.n

    AC.wait_ge(dv.sem, lam_dv)
    AC.activation(lamv["l1"][:], lamv["l1"][:], ACT.Sigmoid, bias=cbias[:, 0:1])
    ac.inc(AC.activation(lamv["l2"][:], lamv["l2"][:], ACT.Sigmoid, bias=cbias[:, 1:2]))
    DV.wait_ge(ac.sem, ac.n)
    DV.tensor_tensor(lamv["lsum"][:], lamv["l1"][:], lamv["l2"][:], op=AOP.add)
    DV.drain()
    DV.tensor_scalar(lamv["lsum"][:], lamv["lsum"][:], 1e-12, None, op0=AOP.max)
    DV.drain()
    DV.reciprocal(lamv["lsum"][:], lamv["lsum"][:])
    DV.drain()
    DV.tensor_tensor(lamv["w0"][:], lamv["l1"][:], lamv["lsum"][:], op=AOP.mult)
    dv.inc(DV.tensor_tensor(lamv["w1"][:], lamv["l2"][:], lamv["lsum"][:],
                            op=AOP.mult))
    w_rdy = dv.n
    w0b = lamv["w0"][:, :, None].broadcast_to([128, NB, 128])
    w1b_ = lamv["w1"][:, :, None].broadcast_to([128, NB, 128])

    def gate_fin(E, lo, hi, fin):
        E.tensor_tensor(agg1[:, lo:hi], agg1[:, lo:hi], w0b[:, lo:hi], op=AOP.mult)
        E.tensor_tensor(agg2[:, lo:hi], agg2[:, lo:hi], w1b_[:, lo:hi], op=AOP.mult)
        E.drain()
        fin(E.tensor_tensor(agg1[:, lo:hi], agg1[:, lo:hi], agg2[:, lo:hi],
                            op=AOP.add))

    GP.wait_ge(dv.sem, w_rdy)
    gate_fin(GP, 0, BDP, gpv.inc)
    xfin_gp = gpv.n
    DV.drain()
    gate_fin(DV, BDP, NB, dv.inc)
    xfin_dv = dv.n

    # L2 table: s2 = x @ W2 (W2 host-padded to 128 cols, cols NCLASS.. zero)
    # transposes use 2 fresh bf16 psum banks; s2 matmuls reuse blk_ps[2:4]
    t3t_ps = [ctx.enter_context(nc.psum_tensor(f"t3t{i}", [128, 128], BF16))
              for i in range(2)]
    s2b_free = {}
    t3_cp = {}

    def t3_mm(t):
        q = blk_ps[2 + t % 2]
        PE.wait_ge(dv.sem, t3_cp[t])
        if t % 2 in s2b_free:
            PE.wait_ge(ac.sem, s2b_free[t % 2])
        else:
            psum_wait(PE, 2 + t % 2)
        pe.inc(PE.matmul(q[:], xtt[:, t % 2, :], c_w2[:], start=True,
                         stop=True))
        AC.wait_ge(pe.sem, pe.n)
        g = t // TBATCH
        if t % TBATCH == 0 and (g % 2) in stg_free:
            fctr, fval = stg_free[g % 2]
            AC.wait_ge(fctr.sem, fval)
        ac.inc(AC.activation(stg[:, (g % 2) * TBATCH + t % TBATCH, :], q[:],
                             ACT.Copy))
        s2b_free[t % 2] = ac.n
        if t % TBATCH == TBATCH - 1 or t == NB - 1:
            b0 = g * TBATCH
            nb = t - b0 + 1
            if g == 0:
                # sgs re-increments must follow cc_t2's pending sgs waits
                SY.wait_ge(ccs[1].sem, cc_done[1])
            SY.wait_ge(ac.sem, ac.n)
            sgs[g % 2].inc(SY.dma_start(
                t_in[3][b0 * 128:(b0 + nb) * 128, :]
                .rearrange("(t p) f -> p t f", p=128),
                stg[:, (g % 2) * TBATCH:(g % 2) * TBATCH + nb, :]))
            stg_free[g % 2] = (sgs[g % 2], sgs[g % 2].n)

    for t in range(NB):
        p = t3t_ps[t % 2]
        if t == 0:
            PE.wait_ge(gpv.sem, xfin_gp)
            PE.wait_ge(aio.sem, caio)
        if t == BDP:
            PE.wait_ge(dv.sem, xfin_dv)
        if t >= 2:
            PE.wait_ge(dv.sem, t3_cp[t - 2])
        pe.inc(PE.transpose(p[:], agg1[:, t, :], c_idb[:]))
        DV.wait_ge(pe.sem, pe.n)
        dv.inc(DV.tensor_copy(xtt[:, t % 2, :], p[:]))
        t3_cp[t] = dv.n
        if t >= 1:
            t3_mm(t - 1)
    t3_mm(NB - 1)
    # hand the reused s2 banks back to the edge passes with last-reader marks
    psum_last[2 + (NB - 1) % 2] = ("ac", s2b_free[(NB - 1) % 2])
    psum_last[2 + (NB - 2) % 2] = ("ac", s2b_free[(NB - 2) % 2])
    cc_start(2, 3, ((sgs[0], sgs[0].n), (sgs[1], sgs[1].n)))

    # =========== L2 edge passes ===========
    edge_pass(1, t_full[3], 64, prop1, 2, m1[0], m1[1], m1[3], m1[4], False)
    edge_pass(2, t_full[3], 64, prop2, 2, m2[0], m2[1], m2[3], m2[4], False)

    # ====== Phase F: final gating, block-split Pool [0,BDP) / DVE [BDP,NB) ======
    b2b = c_b2r[:, None, :].broadcast_to([128, NB, 64])
    h1b_b = c_h1w[:, None, :].broadcast_to([128, NB, 64])
    h2b_b = c_h2w[:, None, :].broadcast_to([128, NB, 64])
    t64 = tmp[:, :, 0:64]

    t64b = tmp2[:, :, 0:64]
    GP.drain()
    GP.tensor_tensor(prop1[:, 0:BDP], prop1[:, 0:BDP], b2b[:, 0:BDP], op=AOP.add)
    gpv.inc(GP.tensor_tensor(prop2[:, 0:BDP], prop2[:, 0:BDP], b2b[:, 0:BDP],
                             op=AOP.add))
    pf_gp = gpv.n
    GP.drain()
    GP.tensor_tensor(t64[:, 0:BDP], prop1[:, 0:BDP], h1b_b[:, 0:BDP], op=AOP.mult)
    gpv.inc(GP.tensor_tensor(t64b[:], prop2[:, 0:BDP], h2b_b[:, 0:BDP],
                             op=AOP.mult))
    mu_gp = gpv.n
    DV.drain()
    DV.tensor_tensor(prop1[:, BDP:NB], prop1[:, BDP:NB], b2b[:, BDP:NB],
                     op=AOP.add)
    dv.inc(DV.tensor_tensor(prop2[:, BDP:NB], prop2[:, BDP:NB], b2b[:, BDP:NB],
                            op=AOP.add))
    pf_dv = dv.n
    DV.drain()
    DV.tensor_tensor(t64[:, BDP:NB], prop1[:, BDP:NB], h1b_b[:, BDP:NB],
                     op=AOP.mult)
    DV.drain()
    DV.tensor_reduce(lamv["l1"][:, BDP:NB], t64[:, BDP:NB], axis=X_AX,
                     op=AOP.add)
    DV.drain()
    DV.tensor_tensor(t64[:, BDP:NB], prop2[:, BDP:NB], h2b_b[:, BDP:NB],
                     op=AOP.mult)
    DV.drain()
    DV.tensor_reduce(lamv["l2"][:, BDP:NB], t64[:, BDP:NB], axis=X_AX,
                     op=AOP.add)
    DV.wait_ge(gpv.sem, mu_gp)
    DV.tensor_reduce(lamv["l1"][:, 0:BDP], t64[:, 0:BDP], axis=X_AX, op=AOP.add)
    dv.inc(DV.tensor_reduce(lamv["l2"][:, 0:BDP], t64b[:], axis=X_AX,
                            op=AOP.add))
    mu_dv = dv.n

    # p1/p2 outputs (biased props are final; Pool software-DGE converts
    # the bf16 SBUF props to the f32 output tensors)
    GP.wait_ge(dv.sem, pf_dv)
    GP.drain()
    pio.inc(GP.dma_start(p1_o[:].rearrange("(t p) f -> p t f", p=128),
                         prop1[:, :, 0:NCLASS]))
    pio.inc(GP.dma_start(p2_o[:].rearrange("(t p) f -> p t f", p=128),
                         prop2[:, :, 0:NCLASS]))
    pout_pio = pio.n

    AC.wait_ge(dv.sem, mu_dv)
    AC.wait_ge(gpv.sem, mu_gp)
    AC.activation(lamv["l1"][:], lamv["l1"][:], ACT.Sigmoid, bias=cbias[:, 2:3])
    ac.inc(AC.activation(lamv["l2"][:], lamv["l2"][:], ACT.Sigmoid, bias=cbias[:, 3:4]))
    DV.wait_ge(ac.sem, ac.n)
    DV.tensor_tensor(lamv["lsum"][:], lamv["l1"][:], lamv["l2"][:], op=AOP.add)
    DV.drain()
    DV.tensor_scalar(lamv["lsum"][:], lamv["lsum"][:], 1e-12, None, op0=AOP.max)
    DV.drain()
    DV.reciprocal(lamv["lsum"][:], lamv["lsum"][:])
    DV.drain()
    DV.tensor_tensor(lamv["w0"][:], lamv["l1"][:], lamv["lsum"][:], op=AOP.mult)
    dv.inc(DV.tensor_tensor(lamv["w1"][:], lamv["l2"][:], lamv["lsum"][:],
                            op=AOP.mult))
    w_rdy2 = dv.n
    w0b6 = lamv["w0"][:, :, None].broadcast_to([128, NB, 64])
    w1b6 = lamv["w1"][:, :, None].broadcast_to([128, NB, 64])

    def fin_out(E, lo, hi, fin):
        E.tensor_tensor(t64[:, lo:hi], prop1[:, lo:hi], w0b6[:, lo:hi], op=AOP.mult)
        E.tensor_tensor(prop2[:, lo:hi], prop2[:, lo:hi], w1b6[:, lo:hi], op=AOP.mult)
        E.drain()
        fin(E.tensor_tensor(t64[:, lo:hi], t64[:, lo:hi], prop2[:, lo:hi],
                            op=AOP.add))

    GP.wait_ge(dv.sem, w_rdy2)
    GP.wait_ge(pio.sem, pout_pio)  # don't clobber props mid-DMA
    fin_out(GP, 0, BDP, gpv.inc)
    out_gp = gpv.n
    DV.drain()
    DV.wait_ge(pio.sem, pout_pio)  # don't clobber props mid-DMA
    fin_out(DV, BDP, NB, dv.inc)
    out_dv = dv.n
    GP.wait_ge(dv.sem, out_dv)
    GP.drain()
    pio.inc(GP.dma_start(out_o[:].rearrange("(t p) f -> p t f", p=128),
                         tmp[:, :, 0:NCLASS]))
    GP.wait_ge(pio.sem, pio.n)
    SY.wait_ge(io.sem, io.n)

    nc.compile()
    ctx.close()
    return nc


def _run(inputs, sim=False):
    S = inputs["x1a"].shape[0] // NCORES
    NB = -(-S // 128)
    SP = NB * 128
    NROWS = NCORES * SP
    HSPLIT = min(32768, NROWS // 2 // 128 * 128)
    node2slot, slot2node = _balance(inputs, SP, NB)

    adj = {}
    adjmeta = {}
    for a in (1, 2):
        out, cpb_lo, cpb_hi, ch_lo, ch_hi, nslot = _prep_adjacency(
            inputs[f"src{a}"], inputs[f"dst{a}"], inputs[f"ew{a}"],
            node2slot, S, SP, NB, HSPLIT, NROWS)
        adj[a] = out
        adjmeta[a] = (ch_lo, ch_hi, nslot, cpb_lo, cpb_hi)

    scalars = (float(np.asarray(inputs["g1b"]).ravel()[0]),
               float(np.asarray(inputs["g2b"]).ravel()[0]),
               float(np.asarray(inputs["h1b"]).ravel()[0]),
               float(np.asarray(inputs["h2b"]).ravel()[0]))
    nc = _build(S, SP, NB, NROWS, HSPLIT, adjmeta, scalars)

    bf = ml_dtypes.bfloat16
    f32 = np.float32

    def wfmt(w):  # [256, 64] -> [128, 2, 64] bf16
        return np.ascontiguousarray(
            np.asarray(w, f32).reshape(2, 128, NHID).transpose(1, 0, 2)).astype(bf)

    w2pad = np.zeros((128, 128), f32)
    w2pad[:, :NCLASS] = np.asarray(inputs["W2"], f32)
    iota = np.tile(np.arange(128, dtype=f32), (128, 1))
    ident = np.eye(128, dtype=f32)
    g1w = np.tile(np.asarray(inputs["g1w"], f32).ravel(), (128, 1))
    g2w = np.tile(np.asarray(inputs["g2w"], f32).ravel(), (128, 1))
    h1w = np.zeros((128, 64), f32)
    h1w[:, :NCLASS] = np.asarray(inputs["h1w"], f32).ravel()
    h2w = np.zeros((128, 64), f32)
    h2w[:, :NCLASS] = np.asarray(inputs["h2w"], f32).ravel()
    b1r = np.tile(np.concatenate([np.asarray(inputs["b1a"], f32).ravel(),
                                  np.asarray(inputs["b1b"], f32).ravel()]), (128, 1))
    b2r = np.zeros((128, 64), f32)
    b2r[:, :NCLASS] = np.asarray(inputs["b2"], f32).ravel()

    common = dict(
        w1a=wfmt(inputs["W1a"]), w1b=wfmt(inputs["W1b"]),
        w2=w2pad.astype(bf), iota=iota.astype(bf), idf=ident,
        idb=ident.astype(bf), g1w=g1w.astype(bf), g2w=g2w.astype(bf),
        h1w=h1w.astype(bf), h2w=h2w.astype(bf),
        b1r=b1r.astype(bf), b2r=b2r.astype(bf))

    def xfmt(x, k):  # shard k by slot map, pad, transpose -> [128, 2, SP] bf16
        idx = slot2node[k * SP:(k + 1) * SP]
        m = idx >= 0
        xp = np.zeros((SP, NFEAT), f32)
        xp[m] = np.asarray(x, f32)[idx[m]]
        xt = xp.T.reshape(2, 128, SP).transpose(1, 0, 2)
        return np.ascontiguousarray(xt).astype(bf)

    in_maps = []
    for k in range(NCORES):
        m = dict(common)
        for v, key in (("xt1a", "x1a"), ("xt1b", "x1b"),
                       ("xt2a", "x2a"), ("xt2b", "x2b")):
            m[v] = xfmt(inputs[key], k)
        for a in (1, 2):
            g, d, e = adj[a][k]
            m[f"gidx{a}"] = g
            m[f"dst{a}"] = d
            m[f"eww{a}"] = e
        in_maps.append(m)

    global LAST_EXEC_NS
    if sim:
        from concourse.bass_interp import MultiCoreSim
        msim = MultiCoreSim(nc, NCORES)
        for k in range(NCORES):
            for name, arr in in_maps[k].items():
                msim.cores[k].tensor(name)[:] = arr
        msim.simulate()
        results = [{nm: msim.cores[k].tensor(nm).copy()
                    for nm in ("out_o", "p1_o", "p2_o")} for k in range(NCORES)]
    else:
        import os
        import time as _time
        trace = bool(os.environ.get("KERNEL_TRACE"))
        r = run_bass_kernel_spmd(nc, in_maps, list(range(NCORES)), trace=trace)
        LAST_EXEC_NS = r.exec_time_ns
        results = r.results
        if os.environ.get("KERNEL_REPEAT"):
            t0 = _time.perf_counter()
            run_bass_kernel_spmd(nc, in_maps, list(range(NCORES)))
            global LAST_WALL2_S
            LAST_WALL2_S = _time.perf_counter() - t0

    outs = []
    for nm in ("out_o", "p1_o", "p2_o"):
        full = np.concatenate([results[k][nm] for k in range(NCORES)],
                              axis=0).astype(np.float32)
        outs.append(np.ascontiguousarray(full[node2slot]))
    return tuple(outs)


LAST_EXEC_NS = None
LAST_WALL2_S = None


def kernel(**inputs):
    return _run(inputs, sim=False)



# revision 168
# speedup vs baseline: 1.0056x; 1.0056x over previous
"""DaGCN on 8 Trainium2 NeuronCores (Bass SPMD).

Strategy (graph/data parallel, nodes sharded 8 ways):
  * Each core owns a 6250-node shard (padded to 6272 = 49*128).
  * Feature transforms s = x @ W run as bf16 PE matmuls on host-transposed
    x shards; the resulting per-shard tables are AllGather'ed so every core
    holds the full [50176, 128] bf16 node-feature tables in its DRAM.
  * Edges are assigned to the core owning dst. Per (adjacency, src-half)
    they are sorted by dst block (128 nodes), each block's run padded to
    whole 128-edge chunks. dma_gather (1024 idxs/call) fetches s[src] as
    bf16 messages, edge-on-partition.
  * segment_sum runs on the TensorEngine: per 128-edge chunk a one-hot
    lhsT [128 edges x 128 dst-cols] holding ew (built on DVE from an iota
    compare) is matmul'ed with the message chunk, accumulating each dst
    block in PSUM. No scatter-add (HW races on duplicate indices).
  * Gating/normalization math runs on DVE/ACT over [128, 49, F] shard
    layouts entirely in SBUF.
"""

import math
from contextlib import ExitStack

import ml_dtypes
import numpy as np

import concourse.bacc as bacc
import concourse.bass as bass
import concourse.mybir as mybir
from concourse.bass_utils import run_bass_kernel_spmd

F32 = mybir.dt.float32
BF16 = mybir.dt.bfloat16
I16 = mybir.dt.int16
AOP = mybir.AluOpType
ACT = mybir.ActivationFunctionType

NCORES = 8
N = 50000
NFEAT, NHID, NCLASS = 256, 64, 32
S_CALL = 1024          # idxs per dma_gather call (HW-validated; 2048 hangs)
CALL_CHUNKS = S_CALL // 128
RING = 8               # gather/onehot ring depth (in calls)
NPSUM = 6              # psum block-accumulator ring


def _wrap16(a):
    """[n] int16 -> [128, n//16]: idx i at [i%16, i//16], replicated x8."""
    n = a.shape[0]
    w = a.reshape(n // 16, 16).T.astype(np.int16)
    return np.tile(w, (8, 1)).copy()


def _chunkwrap(a, dtype):
    """[n] -> [128, n//128]: edge i at [i%128, i//128]."""
    n = a.shape[0]
    return np.ascontiguousarray(a.reshape(n // 128, 128).T.astype(dtype))


def _balance(inputs, SP, NB):
    """Assign nodes to (core, block, col) slots so per-block in-degrees are
    balanced for both adjacencies: deal nodes round-robin (snaking) in
    descending total-degree order, sorting each round by adjacency-1 degree."""
    d1 = np.bincount(np.asarray(inputs["dst1"]).astype(np.int64), minlength=N)
    d2 = np.bincount(np.asarray(inputs["dst2"]).astype(np.int64), minlength=N)
    NBLK = NCORES * NB
    order = np.argsort(-(d1 + d2), kind="stable")
    node2slot = np.full(N, -1, np.int64)
    slot_fill = np.zeros(NBLK, np.int64)
    blocks = np.arange(NBLK)
    pos, r = 0, 0
    while pos < N:
        take = min(NBLK, N - pos)
        grp = order[pos:pos + take]
        grp = grp[np.argsort(-d1[grp], kind="stable")]
        tgt = (blocks if r % 2 == 0 else blocks[::-1])[:take]
        node2slot[grp] = tgt * 128 + slot_fill[tgt]
        slot_fill[tgt] += 1
        pos += take
        r += 1
    slot2node = np.full(NCORES * SP, -1, np.int64)
    slot2node[node2slot] = np.arange(N)
    return node2slot, slot2node


def _prep_adjacency(src, dst, ew, node2slot, S, SP, NB, HSPLIT, NROWS):
    """Bucket edges by dst core/block/src-half; returns per-core arrays + CPBs."""
    src = np.asarray(src).astype(np.int64)
    dst = np.asarray(dst).astype(np.int64)
    ew = np.asarray(ew).astype(np.float32)
    dsl = node2slot[dst]
    core = dsl // SP
    row = node2slot[src]                    # padded table row
    half = (row >= HSPLIT).astype(np.int64)
    dstrel = dsl - core * SP
    blk = dstrel // 128
    col = dstrel % 128

    percore = []
    counts = np.zeros((NCORES, 2, NB), np.int64)
    for k in range(NCORES):
        m = core == k
        e = np.lexsort((blk[m], half[m]))   # sort by (half, block)
        r, h, b, c, w = row[m][e], half[m][e], blk[m][e], col[m][e], ew[m][e]
        percore.append((r, h, b, c, w))
        for hh in range(2):
            mm = h == hh
            counts[k, hh] = np.bincount(b[mm], minlength=NB)

    cpb_lo = int(np.ceil(counts[:, 0].max() / 128))
    cpb_hi = int(np.ceil(counts[:, 1].max() / 128))
    cpb_lo = max(cpb_lo, 1)
    cpb_hi = max(cpb_hi, 1)
    ch_lo = -(-NB * cpb_lo // CALL_CHUNKS) * CALL_CHUNKS
    ch_hi = -(-NB * cpb_hi // CALL_CHUNKS) * CALL_CHUNKS
    nslot = (ch_lo + ch_hi) * 128

    out = []
    for k in range(NCORES):
        r, h, b, c, w = percore[k]
        gidx = np.zeros(nslot, np.int64)
        dcol = np.zeros(nslot, np.int64)
        eww = np.zeros(nslot, np.float32)
        for hh, cpb, base_ch, rowbase in ((0, cpb_lo, 0, 0), (1, cpb_hi, ch_lo, HSPLIT)):
            mm = h == hh
            rr, bb, cc, ww = r[mm], b[mm], c[mm], w[mm]
            # position within block run (edges already sorted by block)
            cnt = counts[k, hh]
            offs = np.concatenate(([0], np.cumsum(cnt)))[:-1]
            pos = np.arange(rr.shape[0]) - offs[bb]
            slot = (base_ch + bb * cpb) * 128 + pos
            gidx[slot] = rr - rowbase
            dcol[slot] = cc
            eww[slot] = ww
        out.append((
            _wrap16(gidx),
            _chunkwrap(dcol, np.float32),
            _chunkwrap(eww, np.float32),
        ))
    return out, cpb_lo, cpb_hi, ch_lo, ch_hi, nslot


class Ctr:
    def __init__(self, sem, step=1):
        self.sem, self.n, self.step = sem, 0, step

    def inc(self, inst):
        inst.then_inc(self.sem, self.step)
        self.n += self.step
        return self.n


def _build(S, SP, NB, NROWS, HSPLIT, adjmeta, scalars):
    """adjmeta: {a: (ch_lo, ch_hi, nslot)}; scalars: g1b,g2b,h1b,h2b floats."""
    nc = bacc.Bacc("TRN2", num_devices=NCORES, num_swdge_queues=2)
    g1b, g2b, h1b, h2b = scalars
    ncal_max = max((m[0] + m[1]) // CALL_CHUNKS for m in adjmeta.values())
    nslot_max = max(m[2] for m in adjmeta.values())
    nch_max = nslot_max // 128

    # ---------------- I/O ----------------
    din = {}
    for v in ("xt1a", "xt1b", "xt2a", "xt2b"):
        din[v] = nc.dram_tensor(v, [128, 2, SP], BF16, kind="ExternalInput")
    din["w1a"] = nc.dram_tensor("w1a", [128, 2, NHID], BF16, kind="ExternalInput")
    din["w1b"] = nc.dram_tensor("w1b", [128, 2, NHID], BF16, kind="ExternalInput")
    din["w2"] = nc.dram_tensor("w2", [128, 128], BF16, kind="ExternalInput")
    din["iota"] = nc.dram_tensor("iota", [128, 128], BF16, kind="ExternalInput")
    din["idf"] = nc.dram_tensor("idf", [128, 128], F32, kind="ExternalInput")
    din["idb"] = nc.dram_tensor("idb", [128, 128], BF16, kind="ExternalInput")
    din["g1w"] = nc.dram_tensor("g1w", [128, 128], BF16, kind="ExternalInput")
    din["g2w"] = nc.dram_tensor("g2w", [128, 128], BF16, kind="ExternalInput")
    din["h1w"] = nc.dram_tensor("h1w", [128, 64], BF16, kind="ExternalInput")
    din["h2w"] = nc.dram_tensor("h2w", [128, 64], BF16, kind="ExternalInput")
    din["b1r"] = nc.dram_tensor("b1r", [128, 128], BF16, kind="ExternalInput")
    din["b2r"] = nc.dram_tensor("b2r", [128, 64], BF16, kind="ExternalInput")
    for a in (1, 2):
        ns = adjmeta[a][2]
        din[f"gidx{a}"] = nc.dram_tensor(f"gidx{a}", [128, ns // 16], I16, kind="ExternalInput")
        din[f"dst{a}"] = nc.dram_tensor(f"dst{a}", [128, ns // 128], F32, kind="ExternalInput")
        din[f"eww{a}"] = nc.dram_tensor(f"eww{a}", [128, ns // 128], F32, kind="ExternalInput")
    out_o = nc.dram_tensor("out_o", [SP, NCLASS], BF16, kind="ExternalOutput")
    p1_o = nc.dram_tensor("p1_o", [SP, NCLASS], BF16, kind="ExternalOutput")
    p2_o = nc.dram_tensor("p2_o", [SP, NCLASS], BF16, kind="ExternalOutput")

    t_in = {t: nc.dram_tensor(f"t{t}in", [SP, 128], BF16) for t in (1, 2, 3)}
    t_full = {t: nc.dram_tensor(f"t{t}full", [NROWS, 128], BF16,
                                addr_space="Shared")
              for t in (1, 2, 3)}

    ctx = ExitStack()
    sb = lambda name, shape, dt: ctx.enter_context(nc.sbuf_tensor(name, shape, dt))
    ps = lambda name, shape: ctx.enter_context(nc.psum_tensor(name, shape, F32))
    sem = lambda name: ctx.enter_context(nc.semaphore(name))

    # ---------------- SBUF ----------------
    c_w1a = sb("c_w1a", [128, 2, NHID], BF16)
    c_w1b = sb("c_w1b", [128, 2, NHID], BF16)
    c_w2 = sb("c_w2", [128, 128], BF16)
    c_iota = sb("c_iota", [128, 128], BF16)
    c_idf = sb("c_idf", [128, 128], F32)
    c_idb = sb("c_idb", [128, 128], BF16)
    c_g1w = sb("c_g1w", [128, 128], BF16)
    c_g2w = sb("c_g2w", [128, 128], BF16)
    c_h1w = sb("c_h1w", [128, 64], BF16)
    c_h2w = sb("c_h2w", [128, 64], BF16)
    c_b1r = sb("c_b1r", [128, 128], BF16)
    c_b2r = sb("c_b2r", [128, 64], BF16)

    stg = sb("stg", [128, 16, 128], BF16)  # table staging: 2 groups of 8 blocks
    agg1 = sb("agg1", [128, NB, 128], BF16)
    agg2 = sb("agg2", [128, NB, 128], BF16)
    tmp = sb("tmp", [128, NB, 128], BF16)
    tmp2 = sb("tmp2", [128, NB - 27, 128], BF16)  # Pool-branch gating scratch
    xtt = sb("xtt", [128, 2, 128], BF16)
    lamv = {nm: sb(nm, [128, NB], F32)
            for nm in ("l1", "l2", "lsum", "w0", "w1")}
    cbias = sb("cbias", [128, 4], F32)
    # per-adjacency edge metadata, loaded once and reused by both layers
    gidx_sb = {a: sb(f"gidx{a}_sb", [128, adjmeta[a][2] // 16], I16)
               for a in (1, 2)}
    dst_sb = {a: sb(f"dst{a}_sb", [128, adjmeta[a][2] // 128], F32)
              for a in (1, 2)}
    ew_sb = {a: sb(f"ew{a}_sb", [128, adjmeta[a][2] // 128], F32)
             for a in (1, 2)}
    sbA = ExitStack()
    xta = sbA.enter_context(nc.sbuf_tensor("xta", [128, 2, SP], BF16))
    xtb = sbA.enter_context(nc.sbuf_tensor("xtb", [128, 2, SP], BF16))
    sT = sbA.enter_context(nc.sbuf_tensor("sT", [128, SP], BF16))

    psA = ExitStack()
    mm_ps = [psA.enter_context(nc.psum_tensor(f"mm_ps{i}", [128, 512], F32))
             for i in range(2)]
    trb_ps = [psA.enter_context(nc.psum_tensor(f"trb_ps{i}", [128, 128], BF16))
              for i in range(2)]

    io = Ctr(sem("io"), 16)        # sync-engine DMAs
    pio = Ctr(sem("pio"), 16)      # Pool software-DGE DMAs
    gpv = Ctr(sem("gpv"), 1)       # Pool compute milestones
    gsems = [Ctr(sem(f"g{i}"), 16) for i in range(RING)]  # per-ring-slot gathers
    ccs = [Ctr(sem(f"cc{i}"), 1) for i in range(3)]   # one sem per collective
    pe = Ctr(sem("pe"), 1)         # PE milestones
    dv = Ctr(sem("dv"), 1)         # DVE milestones
    ac = Ctr(sem("ac"), 1)         # ACT milestones

    SY, PE, DV, AC, GP = nc.sync, nc.tensor, nc.vector, nc.scalar, nc.gpsimd

    def fence():
        # sync engine waits for all its issued DMAs: later cross-engine
        # io-threshold waits become unambiguous (no completion reordering).
        SY.wait_ge(io.sem, io.n)

    # =========== Phase A: constants + s tables ===========
    for bi, bval in enumerate((g1b, g2b, h1b, h2b)):
        nc.vector.memset(cbias[:, bi:bi + 1], float(bval))
    # only the matmul/transpose constants block the critical path; the rest
    # load on the ACT ring after the x pumps (see below)
    for name, t in (("w1a", c_w1a), ("w1b", c_w1b), ("idb", c_idb)):
        io.inc(SY.dma_start(t[:], din[name][:]))
    consts_io = io.n

    nsl = [(j * 512, min(512, SP - j * 512)) for j in range((SP + 511) // 512)]
    trb_free = {}   # trb bank -> dv val of the copy that freed it
    stg_free = {}   # stg group -> pio val of the batched DMA that freed it
    mmps_free = {}  # mm_ps bank -> dv val of the sT copy that freed it
    aio = Ctr(sem("aio"), 16)   # ACT-ring DMAs (idx loads)
    TBATCH = 8                  # t_in rows per batched DMA (stg is 2 groups)
    # rotating DMA-completion sems: same-sem updates are spaced far apart, so
    # intermediate-threshold waits are deterministic (no overlapping windows)
    NXS = 6
    xsa = [Ctr(sem(f"xsa{i}"), 16) for i in range(NXS)]   # xta slabs (SY)
    xsb = [Ctr(sem(f"xsb{i}"), 16) for i in range(NXS)]   # xtb slabs (ACT)
    sgs = [Ctr(sem(f"sgs{i}"), 16) for i in range(2)]     # t2/t3 flushes (hwdge)
    sgp = [Ctr(sem(f"sgp{i}"), 16) for i in range(2)]     # t1 flushes (Pool swdge)

    def s_table(tbl, va, vb, wa, wb, prev_mm, fe, fs, pre_flush=None):
        """s = [x_va@W1a | x_vb@W1b] -> row-major t_in[tbl], pipelined:
        per-slab x DMAs (xta on SY ring, xtb on ACT ring) -> PE matmul ->
        DVE copy to sT -> per-block PE transpose -> DVE copy into stg ->
        batched row DMA (SY). The x pump runs NXS slabs ahead of PE so each
        slab-class sem is re-incremented only after its waiter passed."""
        xio = [None] * len(nsl)
        mm = []
        copies = []

        def pump(j):
            o, n = nsl[j]
            k = j % NXS
            if j >= NXS:
                SY.wait_ge(pe.sem, mm[j - NXS])
                AC.wait_ge(pe.sem, mm[j - NXS])
            xsa[k].inc(SY.dma_start(xta[:, :, o:o + n], din[va][:, :, o:o + n]))
            xsb[k].inc(AC.dma_start(xtb[:, :, o:o + n], din[vb][:, :, o:o + n]))
            xio[j] = (k, xsa[k].n, xsb[k].n)

        if prev_mm is not None:
            SY.wait_ge(pe.sem, prev_mm[-1])   # xta/xtb + class-sem WAR
            AC.wait_ge(pe.sem, prev_mm[-1])
        for j in range(min(NXS, len(nsl))):
            pump(j)

        def flush_group(t_last):
            """DMA stg group (blocks g*TBATCH..t_last) -> t_in rows."""
            g = t_last // TBATCH
            b0 = g * TBATCH
            nb = t_last - b0 + 1
            s0 = (g % 2) * TBATCH
            if pre_flush is not None and g == 0:
                pre_flush()
            fe.wait_ge(dv.sem, dv.n)
            fs[g % 2].inc(fe.dma_start(
                t_in[tbl][b0 * 128:(b0 + nb) * 128, :]
                .rearrange("(t p) f -> p t f", p=128),
                stg[:, s0:s0 + nb, :]))
            stg_free[g % 2] = (fs[g % 2], fs[g % 2].n)

        def do_blocks(jj):
            for t in range(jj * 4, min((jj + 1) * 4, NB)):
                q = trb_ps[t % 2]
                PE.wait_ge(dv.sem, copies[jj])
                if t % 2 in trb_free:
                    PE.wait_ge(dv.sem, trb_free[t % 2])
                pe.inc(PE.transpose(q[:], sT[:, t * 128:(t + 1) * 128],
                                    c_idb[:]))
                DV.wait_ge(pe.sem, pe.n)
                g = t // TBATCH
                if t % TBATCH == 0 and (g % 2) in stg_free:
                    fctr, fval = stg_free[g % 2]
                    DV.wait_ge(fctr.sem, fval)
                dv.inc(DV.tensor_copy(
                    stg[:, (g % 2) * TBATCH + t % TBATCH, :], q[:]))
                trb_free[t % 2] = dv.n
                if t % TBATCH == TBATCH - 1 or t == NB - 1:
                    flush_group(t)

        for j, (o, n) in enumerate(nsl):
            p = mm_ps[j % 2]
            k, va_, vb_ = xio[j]
            PE.wait_ge(xsa[k].sem, va_)
            PE.wait_ge(xsb[k].sem, vb_)
            if j % 2 in mmps_free:
                PE.wait_ge(dv.sem, mmps_free[j % 2])
            for xt, w, prow in ((xta, wa, 0), (xtb, wb, 64)):
                for cch in range(2):
                    last = PE.matmul(p[prow:prow + 64, 0:n], w[:, cch, :],
                                     xt[:, cch, o:o + n],
                                     start=(cch == 0), stop=(cch == 1))
            pe.inc(last)
            mm.append(pe.n)
            if j + NXS < len(nsl):
                pump(j + NXS)
            DV.wait_ge(pe.sem, pe.n)
            dv.inc(DV.tensor_copy(sT[:, o:o + n], p[:, 0:n]))
            copies.append(dv.n)
            mmps_free[j % 2] = dv.n
            if j >= 1:
                do_blocks(j - 1)
        do_blocks(len(nsl) - 1)
        return ((fs[0], fs[0].n), (fs[1], fs[1].n)), mm

    CCPAGES = 4096  # factor the (contiguous) collective out AP into pages
    cc_done = {}

    def cc_start(ci, t, wait_sgs):
        """AllGather t_in[t] -> t_full[t] on the Pool engine. The out AP is
        re-factored into [pages, page] pairs — the identical contiguous
        region, expressed page-wise."""
        for fctr, fval in wait_sgs:
            GP.wait_ge(fctr.sem, fval)
        cc = GP.collective_compute(
            "AllGather", AOP.bypass,
            replica_groups=[list(range(NCORES))],
            ins=[t_in[t][:]], outs=[t_full[t][:]])
        per = NROWS * 128 // CCPAGES
        cc.ins.outs[0].ap = [[per, CCPAGES], [1, per]]
        ccs[ci].inc(cc)
        cc_done[ci] = ccs[ci].n

    # edge metadata: adjacency 1 on Pool right after cc_t1 (its idle window);
    # adjacency 2 on the SY ring once the x pumps are done
    idx_wait = {}

    def load_idx(a, eng, ctr):
        ctr.inc(eng.dma_start(gidx_sb[a][:], din[f"gidx{a}"][:]))
        ctr.inc(eng.dma_start(dst_sb[a][:], din[f"dst{a}"][:]))
        ctr.inc(eng.dma_start(ew_sb[a][:], din[f"eww{a}"][:]))
        idx_wait[a] = (ctr, ctr.n)

    PE.wait_ge(io.sem, consts_io)   # w1a/w1b/idb loaded
    t1_io, t1_mm = s_table(1, "xt1a", "xt1b", c_w1a, c_w1b, None, GP, sgp)
    cc_start(0, 1, t1_io)
    load_idx(1, GP, pio)
    t2_io, t2_mm = s_table(2, "xt2a", "xt2b", c_w1a, c_w1b, t1_mm, AC, sgs)
    load_idx(2, SY, io)
    # remaining constants on the ACT ring, clear of the pumps
    for name, t in (("iota", c_iota), ("idf", c_idf), ("w2", c_w2),
                    ("g1w", c_g1w), ("g2w", c_g2w), ("h1w", c_h1w),
                    ("h2w", c_h2w), ("b1r", c_b1r), ("b2r", c_b2r)):
        aio.inc(AC.dma_start(t[:], din[name][:]))
    caio = aio.n
    DV.wait_ge(aio.sem, caio)   # c_iota before the first onehot build

    pe_phaseA = pe.n

    # =========== edge pass machinery ===========
    psA.close()  # phase-A PSUM freed; per-engine program order makes reuse safe
    sbA.close()  # xta/xtb freed -> reused by edge buffers (guarded by waits below)
    blk_ps = [ps(f"blk_ps{i}", [128, 128]) for i in range(NPSUM)]
    prop1 = sb("prop1", [128, NB, 64], BF16)
    prop2 = sb("prop2", [128, NB, 64], BF16)
    msg = sb("msg", [128, RING * CALL_CHUNKS, 128], BF16)
    ohr = sb("ohr", [128, RING * CALL_CHUNKS, 128], BF16)
    hstage = sb("hstage", [128, NPSUM, 128], BF16)  # hi-half psum staging
    gcall = [0]      # global gather call counter
    pe_cons_vals = []
    npass = [0]
    psum_last = [None] * NPSUM  # (sem, val) of last copy freeing each psum slot
    BDP = NB - 27    # blocks [0,BDP) combine on Pool; [BDP,NB) on DVE
    hst_free = {}    # hstage slot -> gpv val of the Pool add that freed it

    def psum_wait(E, slot):
        if psum_last[slot] is not None:
            eng, val = psum_last[slot]
            E.wait_ge({"dv": dv, "ac": ac, "gp": gpv}[eng].sem, val)

    def edge_pass(adj, table, F_rhs, dest, cc_need, ch_lo, ch_hi, cpb_lo, cpb_hi,
                  add_mode, inject=None):
        """One (layer, adjacency) pass: lo half then hi half.
        Returns per-block dv marks (hi-half add of block b complete)."""
        gbuf = msg
        gx, dx, ex = gidx_sb[adj], dst_sb[adj], ew_sb[adj]
        local_call = [0]
        deferred = []    # (block, psum slot, ac stage val, call idx)

        def flush_deferred(min_age):
            while deferred and gcall[0] - deferred[0][3] >= min_age:
                b, slot, av, _ = deferred.pop(0)
                GP.wait_ge(ac.sem, av)   # also covers lo_copy_ac[b] (same stream)
                gpv.inc(GP.tensor_tensor(dest[:, b, 0:F_rhs],
                                         dest[:, b, 0:F_rhs],
                                         hstage[:, slot, 0:F_rhs], op=AOP.add))
                hst_free[slot] = ("gp", gpv.n)

        lo_copy_ac = {}
        GP.wait_ge(ccs[cc_need].sem, cc_done[cc_need])
        ictr, ival = idx_wait[adj]
        GP.wait_ge(ictr.sem, ival)
        DV.wait_ge(ictr.sem, ival)
        if npass[0] == 0:
            GP.wait_ge(pe.sem, pe_phaseA)   # msg ring aliases freed xta/xtb
            DV.wait_ge(pe.sem, pe_phaseA)   # ohr ring likewise
        npass[0] += 1
        for half, ch, cpb, base in ((0, ch_lo, cpb_lo, 0), (1, ch_hi, cpb_hi, HSPLIT)):
            ch0 = 0 if half == 0 else ch_lo  # chunk offset in the arrays
            tab = table[base:NROWS] if half == 1 else table[0:HSPLIT]
            blk_of = lambda c: min(c // cpb, NB - 1)
            endc = lambda b: (b + 1) * cpb - 1 if b < NB - 1 else ch - 1
            for j in range(ch // CALL_CHUNKS):
                if inject and local_call[0] in inject:
                    inject[local_call[0]]()
                local_call[0] += 1
                rj = (gcall[0] % RING) * CALL_CHUNKS
                flush_deferred(6)
                if len(pe_cons_vals) >= RING:
                    GP.wait_ge(pe.sem, pe_cons_vals[-RING])
                gslot = gcall[0] % RING
                g = GP.dma_gather(
                    gbuf[:, rj:rj + CALL_CHUNKS, :], tab,
                    gx[:, (ch0 * 8 + j * S_CALL // 16):(ch0 * 8 + (j + 1) * S_CALL // 16)],
                    S_CALL, S_CALL, 128, queue_num=gcall[0] % 2)
                gsems[gslot].inc(g)
                gv = gsems[gslot].n
                # onehot build
                if len(pe_cons_vals) >= RING:
                    DV.wait_ge(pe.sem, pe_cons_vals[-RING])
                cbase = ch0 + j * CALL_CHUNKS
                for c8 in range(CALL_CHUNKS):
                    ts = DV.tensor_scalar(
                        ohr[:, rj + c8, :], c_iota[:],
                        dx[:, cbase + c8:cbase + c8 + 1],
                        ex[:, cbase + c8:cbase + c8 + 1],
                        op0=AOP.is_equal, op1=AOP.mult)
                dv.inc(ts)
                ohv = dv.n
                # matmuls
                PE.wait_ge(gsems[gslot].sem, gv)
                PE.wait_ge(dv.sem, ohv)
                last_was_end = False
                for c8 in range(CALL_CHUNKS):
                    c = j * CALL_CHUNKS + c8
                    b = blk_of(c)
                    slot = b % NPSUM
                    p = blk_ps[slot]
                    st = (c == b * cpb)
                    if st:
                        psum_wait(PE, slot)
                    mmi = PE.matmul(p[:, 0:F_rhs],
                                    ohr[:, rj + c8, :],
                                    gbuf[:, rj + c8, 0:F_rhs],
                                    start=st, stop=(c == endc(b)))
                    last_was_end = (c == endc(b))
                    if last_was_end:
                        pe.inc(mmi)
                        if (add_mode or half == 1) and b < BDP:
                            # ACT stages the psum out; Pool combines (deferred)
                            AC.wait_ge(pe.sem, pe.n)
                            if slot in hst_free:
                                heng, hval = hst_free[slot]
                                AC.wait_ge({"gp": gpv, "dv": dv}[heng].sem,
                                           hval)
                            ac.inc(AC.activation(hstage[:, slot, 0:F_rhs],
                                                 p[:, 0:F_rhs], ACT.Copy))
                            psum_last[slot] = ("ac", ac.n)
                            deferred.append((b, slot, ac.n, gcall[0]))
                        elif add_mode or half == 1:
                            DV.wait_ge(pe.sem, pe.n)
                            DV.wait_ge(ac.sem, lo_copy_ac[b])
                            cpi = DV.tensor_tensor(dest[:, b, 0:F_rhs],
                                                   dest[:, b, 0:F_rhs],
                                                   p[:, 0:F_rhs], op=AOP.add)
                            dv.inc(cpi)
                            psum_last[slot] = ("dv", dv.n)
                        else:
                            AC.wait_ge(pe.sem, pe.n)
                            cpi = AC.activation(dest[:, b, 0:F_rhs],
                                                p[:, 0:F_rhs], ACT.Copy)
                            ac.inc(cpi)
                            psum_last[slot] = ("ac", ac.n)
                            lo_copy_ac[b] = ac.n
                if not last_was_end:
                    pe.inc(mmi)
                pe_cons_vals.append(pe.n)
                gcall[0] += 1
        flush_deferred(0)
        return gpv.n

    m1 = adjmeta[1]
    m2 = adjmeta[2]
    inj1 = {18: lambda: cc_start(1, 2, t2_io)}
    edge_pass(1, t_full[1], 128, agg1, 0, m1[0], m1[1], m1[3], m1[4], False,
              inject=inj1)
    edge_pass(2, t_full[2], 128, agg2, 1, m2[0], m2[1], m2[3], m2[4], False)

    # ====== Phase C: mid gating, block-split Pool [0,BDP) / DVE [BDP,NB) ======
    b1b = c_b1r[:, None, :].broadcast_to([128, NB, 128])
    g1b_b = c_g1w[:, None, :].broadcast_to([128, NB, 128])
    g2b_b = c_g2w[:, None, :].broadcast_to([128, NB, 128])
    X_AX = mybir.AxisListType.X

    GP.wait_ge(aio.sem, caio)
    GP.drain()
    GP.tensor_tensor(agg1[:, 0:BDP], agg1[:, 0:BDP], b1b[:, 0:BDP], op=AOP.add)
    GP.tensor_tensor(agg2[:, 0:BDP], agg2[:, 0:BDP], b1b[:, 0:BDP], op=AOP.add)
    GP.drain()
    GP.tensor_scalar(agg1[:, 0:BDP], agg1[:, 0:BDP], 0.0, None, op0=AOP.max)
    GP.tensor_scalar(agg2[:, 0:BDP], agg2[:, 0:BDP], 0.0, None, op0=AOP.max)
    GP.drain()
    GP.tensor_tensor(tmp[:, 0:BDP], agg1[:, 0:BDP], g1b_b[:, 0:BDP], op=AOP.mult)
    gpv.inc(GP.tensor_tensor(tmp2[:], agg2[:, 0:BDP], g2b_b[:, 0:BDP],
                             op=AOP.mult))
    gp_prod = gpv.n
    DV.drain()
    DV.tensor_tensor(agg1[:, BDP:NB], agg1[:, BDP:NB], b1b[:, BDP:NB], op=AOP.add)
    DV.tensor_tensor(agg2[:, BDP:NB], agg2[:, BDP:NB], b1b[:, BDP:NB], op=AOP.add)
    DV.drain()
    DV.tensor_scalar(agg1[:, BDP:NB], agg1[:, BDP:NB], 0.0, None, op0=AOP.max)
    DV.tensor_scalar(agg2[:, BDP:NB], agg2[:, BDP:NB], 0.0, None, op0=AOP.max)
    DV.drain()
    DV.tensor_tensor(tmp[:, BDP:NB], agg1[:, BDP:NB], g1b_b[:, BDP:NB],
                     op=AOP.mult)
    DV.drain()
    DV.tensor_reduce(lamv["l1"][:, BDP:NB], tmp[:, BDP:NB], axis=X_AX,
                     op=AOP.add)
    DV.drain()
    DV.tensor_tensor(tmp[:, BDP:NB], agg2[:, BDP:NB], g2b_b[:, BDP:NB],
                     op=AOP.mult)
    DV.drain()
    DV.tensor_reduce(lamv["l2"][:, BDP:NB], tmp[:, BDP:NB], axis=X_AX,
                     op=AOP.add)
    DV.wait_ge(gpv.sem, gp_prod)
    DV.tensor_reduce(lamv["l1"][:, 0:BDP], tmp[:, 0:BDP], axis=X_AX, op=AOP.add)
    dv.inc(DV.tensor_reduce(lamv["l2"][:, 0:BDP], tmp2[:], axis=X_AX,
                            op=AOP.add))
    lam_dv = dv.n

    AC.wait_ge(dv.sem, lam_dv)
    AC.activation(lamv["l1"][:], lamv["l1"][:], ACT.Sigmoid, bias=cbias[:, 0:1])
    ac.inc(AC.activation(lamv["l2"][:], lamv["l2"][:], ACT.Sigmoid, bias=cbias[:, 1:2]))
    DV.wait_ge(ac.sem, ac.n)
    DV.tensor_tensor(lamv["lsum"][:], lamv["l1"][:], lamv["l2"][:], op=AOP.add)
    DV.drain()
    DV.tensor_scalar(lamv["lsum"][:], lamv["lsum"][:], 1e-12, None, op0=AOP.max)
    DV.drain()
    DV.reciprocal(lamv["lsum"][:], lamv["lsum"][:])
    DV.drain()
    DV.tensor_tensor(lamv["w0"][:], lamv["l1"][:], lamv["lsum"][:], op=AOP.mult)
    dv.inc(DV.tensor_tensor(lamv["w1"][:], lamv["l2"][:], lamv["lsum"][:],
                            op=AOP.mult))
    w_rdy = dv.n
    w0b = lamv["w0"][:, :, None].broadcast_to([128, NB, 128])
    w1b_ = lamv["w1"][:, :, None].broadcast_to([128, NB, 128])

    def gate_fin(E, lo, hi, fin):
        E.tensor_tensor(agg1[:, lo:hi], agg1[:, lo:hi], w0b[:, lo:hi], op=AOP.mult)
        E.tensor_tensor(agg2[:, lo:hi], agg2[:, lo:hi], w1b_[:, lo:hi], op=AOP.mult)
        E.drain()
        fin(E.tensor_tensor(agg1[:, lo:hi], agg1[:, lo:hi], agg2[:, lo:hi],
                            op=AOP.add))

    GP.wait_ge(dv.sem, w_rdy)
    gate_fin(GP, 0, BDP, gpv.inc)
    xfin_gp = gpv.n
    DV.drain()
    gate_fin(DV, BDP, NB, dv.inc)
    xfin_dv = dv.n

    # L2 table: s2 = x @ W2 (W2 host-padded to 128 cols, cols NCLASS.. zero)
    # transposes use 2 fresh bf16 psum banks; s2 matmuls reuse blk_ps[2:4]
    t3t_ps = [ctx.enter_context(nc.psum_tensor(f"t3t{i}", [128, 128], BF16))
              for i in range(2)]
    s2b_free = {}
    t3_cp = {}

    def t3_mm(t):
        q = blk_ps[2 + t % 2]
        PE.wait_ge(dv.sem, t3_cp[t])
        if t % 2 in s2b_free:
            PE.wait_ge(ac.sem, s2b_free[t % 2])
        else:
            psum_wait(PE, 2 + t % 2)
        pe.inc(PE.matmul(q[:], xtt[:, t % 2, :], c_w2[:], start=True,
                         stop=True))
        AC.wait_ge(pe.sem, pe.n)
        g = t // TBATCH
        if t % TBATCH == 0 and (g % 2) in stg_free:
            fctr, fval = stg_free[g % 2]
            AC.wait_ge(fctr.sem, fval)
        ac.inc(AC.activation(stg[:, (g % 2) * TBATCH + t % TBATCH, :], q[:],
                             ACT.Copy))
        s2b_free[t % 2] = ac.n
        if t % TBATCH == TBATCH - 1 or t == NB - 1:
            b0 = g * TBATCH
            nb = t - b0 + 1
            if g == 0:
                # sgs re-increments must follow cc_t2's pending sgs waits
                SY.wait_ge(ccs[1].sem, cc_done[1])
            SY.wait_ge(ac.sem, ac.n)
            sgs[g % 2].inc(SY.dma_start(
                t_in[3][b0 * 128:(b0 + nb) * 128, :]
                .rearrange("(t p) f -> p t f", p=128),
                stg[:, (g % 2) * TBATCH:(g % 2) * TBATCH + nb, :]))
            stg_free[g % 2] = (sgs[g % 2], sgs[g % 2].n)

    for t in range(NB):
        p = t3t_ps[t % 2]
        if t == 0:
            PE.wait_ge(gpv.sem, xfin_gp)
            PE.wait_ge(aio.sem, caio)
        if t == BDP:
            PE.wait_ge(dv.sem, xfin_dv)
        if t >= 2:
            PE.wait_ge(dv.sem, t3_cp[t - 2])
        pe.inc(PE.transpose(p[:], agg1[:, t, :], c_idb[:]))
        DV.wait_ge(pe.sem, pe.n)
        dv.inc(DV.tensor_copy(xtt[:, t % 2, :], p[:]))
        t3_cp[t] = dv.n
        if t >= 1:
            t3_mm(t - 1)
    t3_mm(NB - 1)
    # hand the reused s2 banks back to the edge passes with last-reader marks
    psum_last[2 + (NB - 1) % 2] = ("ac", s2b_free[(NB - 1) % 2])
    psum_last[2 + (NB - 2) % 2] = ("ac", s2b_free[(NB - 2) % 2])
    cc_start(2, 3, ((sgs[0], sgs[0].n), (sgs[1], sgs[1].n)))

    # =========== L2 edge passes ===========
    edge_pass(1, t_full[3], 64, prop1, 2, m1[0], m1[1], m1[3], m1[4], False)
    edge_pass(2, t_full[3], 64, prop2, 2, m2[0], m2[1], m2[3], m2[4], False)

    # ====== Phase F: final gating, block-split Pool [0,BDP) / DVE [BDP,NB) ======
    b2b = c_b2r[:, None, :].broadcast_to([128, NB, 64])
    h1b_b = c_h1w[:, None, :].broadcast_to([128, NB, 64])
    h2b_b = c_h2w[:, None, :].broadcast_to([128, NB, 64])
    t64 = tmp[:, :, 0:64]

    t64b = tmp2[:, :, 0:64]
    GP.drain()
    GP.tensor_tensor(prop1[:, 0:BDP], prop1[:, 0:BDP], b2b[:, 0:BDP], op=AOP.add)
    gpv.inc(GP.tensor_tensor(prop2[:, 0:BDP], prop2[:, 0:BDP], b2b[:, 0:BDP],
                             op=AOP.add))
    pf_gp = gpv.n
    GP.drain()
    GP.tensor_tensor(t64[:, 0:BDP], prop1[:, 0:BDP], h1b_b[:, 0:BDP], op=AOP.mult)
    gpv.inc(GP.tensor_tensor(t64b[:], prop2[:, 0:BDP], h2b_b[:, 0:BDP],
                             op=AOP.mult))
    mu_gp = gpv.n
    DV.drain()
    DV.tensor_tensor(prop1[:, BDP:NB], prop1[:, BDP:NB], b2b[:, BDP:NB],
                     op=AOP.add)
    dv.inc(DV.tensor_tensor(prop2[:, BDP:NB], prop2[:, BDP:NB], b2b[:, BDP:NB],
                            op=AOP.add))
    pf_dv = dv.n
    DV.drain()
    DV.tensor_tensor(t64[:, BDP:NB], prop1[:, BDP:NB], h1b_b[:, BDP:NB],
                     op=AOP.mult)
    DV.drain()
    DV.tensor_reduce(lamv["l1"][:, BDP:NB], t64[:, BDP:NB], axis=X_AX,
                     op=AOP.add)
    DV.drain()
    DV.tensor_tensor(t64[:, BDP:NB], prop2[:, BDP:NB], h2b_b[:, BDP:NB],
                     op=AOP.mult)
    DV.drain()
    DV.tensor_reduce(lamv["l2"][:, BDP:NB], t64[:, BDP:NB], axis=X_AX,
                     op=AOP.add)
    DV.wait_ge(gpv.sem, mu_gp)
    DV.tensor_reduce(lamv["l1"][:, 0:BDP], t64[:, 0:BDP], axis=X_AX, op=AOP.add)
    dv.inc(DV.tensor_reduce(lamv["l2"][:, 0:BDP], t64b[:], axis=X_AX,
                            op=AOP.add))
    mu_dv = dv.n

    # p1/p2 outputs (biased props are final; Pool software-DGE converts
    # the bf16 SBUF props to the f32 output tensors)
    SY.wait_ge(dv.sem, pf_dv)
    SY.wait_ge(gpv.sem, pf_gp)
    io.inc(SY.dma_start(p1_o[:].rearrange("(t p) f -> p t f", p=128),
                        prop1[:, :, 0:NCLASS]))
    io.inc(SY.dma_start(p2_o[:].rearrange("(t p) f -> p t f", p=128),
                        prop2[:, :, 0:NCLASS]))
    fence()
    pout_io = io.n

    AC.wait_ge(dv.sem, mu_dv)
    AC.wait_ge(gpv.sem, mu_gp)
    AC.activation(lamv["l1"][:], lamv["l1"][:], ACT.Sigmoid, bias=cbias[:, 2:3])
    ac.inc(AC.activation(lamv["l2"][:], lamv["l2"][:], ACT.Sigmoid, bias=cbias[:, 3:4]))
    DV.wait_ge(ac.sem, ac.n)
    DV.tensor_tensor(lamv["lsum"][:], lamv["l1"][:], lamv["l2"][:], op=AOP.add)
    DV.drain()
    DV.tensor_scalar(lamv["lsum"][:], lamv["lsum"][:], 1e-12, None, op0=AOP.max)
    DV.drain()
    DV.reciprocal(lamv["lsum"][:], lamv["lsum"][:])
    DV.drain()
    DV.tensor_tensor(lamv["w0"][:], lamv["l1"][:], lamv["lsum"][:], op=AOP.mult)
    dv.inc(DV.tensor_tensor(lamv["w1"][:], lamv["l2"][:], lamv["lsum"][:],
                            op=AOP.mult))
    w_rdy2 = dv.n
    w0b6 = lamv["w0"][:, :, None].broadcast_to([128, NB, 64])
    w1b6 = lamv["w1"][:, :, None].broadcast_to([128, NB, 64])

    def fin_out(E, lo, hi, fin):
        E.tensor_tensor(t64[:, lo:hi], prop1[:, lo:hi], w0b6[:, lo:hi], op=AOP.mult)
        E.tensor_tensor(prop2[:, lo:hi], prop2[:, lo:hi], w1b6[:, lo:hi], op=AOP.mult)
        E.drain()
        fin(E.tensor_tensor(t64[:, lo:hi], t64[:, lo:hi], prop2[:, lo:hi],
                            op=AOP.add))

    GP.wait_ge(dv.sem, w_rdy2)
    GP.wait_ge(io.sem, pout_io)  # don't clobber props mid-DMA
    fin_out(GP, 0, BDP, gpv.inc)
    out_gp = gpv.n
    DV.drain()
    DV.wait_ge(io.sem, pout_io)  # don't clobber props mid-DMA
    fin_out(DV, BDP, NB, dv.inc)
    out_dv = dv.n
    SY.wait_ge(dv.sem, out_dv)
    SY.wait_ge(gpv.sem, out_gp)
    io.inc(SY.dma_start(out_o[:].rearrange("(t p) f -> p t f", p=128),
                        tmp[:, :, 0:NCLASS]))
    SY.wait_ge(io.sem, io.n)
    GP.wait_ge(pio.sem, pio.n)

    nc.compile()
    ctx.close()
    return nc


def _run(inputs, sim=False):
    S = inputs["x1a"].shape[0] // NCORES
    NB = -(-S // 128)
    SP = NB * 128
    NROWS = NCORES * SP
    HSPLIT = min(32768, NROWS // 2 // 128 * 128)
    node2slot, slot2node = _balance(inputs, SP, NB)

    adj = {}
    adjmeta = {}
    for a in (1, 2):
        out, cpb_lo, cpb_hi, ch_lo, ch_hi, nslot = _prep_adjacency(
            inputs[f"src{a}"], inputs[f"dst{a}"], inputs[f"ew{a}"],
            node2slot, S, SP, NB, HSPLIT, NROWS)
        adj[a] = out
        adjmeta[a] = (ch_lo, ch_hi, nslot, cpb_lo, cpb_hi)

    scalars = (float(np.asarray(inputs["g1b"]).ravel()[0]),
               float(np.asarray(inputs["g2b"]).ravel()[0]),
               float(np.asarray(inputs["h1b"]).ravel()[0]),
               float(np.asarray(inputs["h2b"]).ravel()[0]))
    nc = _build(S, SP, NB, NROWS, HSPLIT, adjmeta, scalars)

    bf = ml_dtypes.bfloat16
    f32 = np.float32

    def wfmt(w):  # [256, 64] -> [128, 2, 64] bf16
        return np.ascontiguousarray(
            np.asarray(w, f32).reshape(2, 128, NHID).transpose(1, 0, 2)).astype(bf)

    w2pad = np.zeros((128, 128), f32)
    w2pad[:, :NCLASS] = np.asarray(inputs["W2"], f32)
    iota = np.tile(np.arange(128, dtype=f32), (128, 1))
    ident = np.eye(128, dtype=f32)
    g1w = np.tile(np.asarray(inputs["g1w"], f32).ravel(), (128, 1))
    g2w = np.tile(np.asarray(inputs["g2w"], f32).ravel(), (128, 1))
    h1w = np.zeros((128, 64), f32)
    h1w[:, :NCLASS] = np.asarray(inputs["h1w"], f32).ravel()
    h2w = np.zeros((128, 64), f32)
    h2w[:, :NCLASS] = np.asarray(inputs["h2w"], f32).ravel()
    b1r = np.tile(np.concatenate([np.asarray(inputs["b1a"], f32).ravel(),
                                  np.asarray(inputs["b1b"], f32).ravel()]), (128, 1))
    b2r = np.zeros((128, 64), f32)
    b2r[:, :NCLASS] = np.asarray(inputs["b2"], f32).ravel()

    common = dict(
        w1a=wfmt(inputs["W1a"]), w1b=wfmt(inputs["W1b"]),
        w2=w2pad.astype(bf), iota=iota.astype(bf), idf=ident,
        idb=ident.astype(bf), g1w=g1w.astype(bf), g2w=g2w.astype(bf),
        h1w=h1w.astype(bf), h2w=h2w.astype(bf),
        b1r=b1r.astype(bf), b2r=b2r.astype(bf))

    def xfmt(x, k):  # shard k by slot map, pad, transpose -> [128, 2, SP] bf16
        idx = slot2node[k * SP:(k + 1) * SP]
        m = idx >= 0
        xp = np.zeros((SP, NFEAT), f32)
        xp[m] = np.asarray(x, f32)[idx[m]]
        xt = xp.T.reshape(2, 128, SP).transpose(1, 0, 2)
        return np.ascontiguousarray(xt).astype(bf)

    in_maps = []
    for k in range(NCORES):
        m = dict(common)
        for v, key in (("xt1a", "x1a"), ("xt1b", "x1b"),
                       ("xt2a", "x2a"), ("xt2b", "x2b")):
            m[v] = xfmt(inputs[key], k)
        for a in (1, 2):
            g, d, e = adj[a][k]
            m[f"gidx{a}"] = g
            m[f"dst{a}"] = d
            m[f"eww{a}"] = e
        in_maps.append(m)

    global LAST_EXEC_NS
    if sim:
        from concourse.bass_interp import MultiCoreSim
        msim = MultiCoreSim(nc, NCORES)
        for k in range(NCORES):
            for name, arr in in_maps[k].items():
                msim.cores[k].tensor(name)[:] = arr
        msim.simulate()
        results = [{nm: msim.cores[k].tensor(nm).copy()
                    for nm in ("out_o", "p1_o", "p2_o")} for k in range(NCORES)]
    else:
        import os
        import time as _time
        trace = bool(os.environ.get("KERNEL_TRACE"))
        r = run_bass_kernel_spmd(nc, in_maps, list(range(NCORES)), trace=trace)
        LAST_EXEC_NS = r.exec_time_ns
        results = r.results
        if os.environ.get("KERNEL_REPEAT"):
            t0 = _time.perf_counter()
            run_bass_kernel_spmd(nc, in_maps, list(range(NCORES)))
            global LAST_WALL2_S
            LAST_WALL2_S = _time.perf_counter() - t0

    outs = []
    for nm in ("out_o", "p1_o", "p2_o"):
        full = np.concatenate([results[k][nm] for k in range(NCORES)],
                              axis=0).astype(np.float32)
        outs.append(np.ascontiguousarray(full[node2slot]))
    return tuple(outs)


LAST_EXEC_NS = None
LAST_WALL2_S = None


def kernel(**inputs):
    return _run(inputs, sim=False)

